# revision 39
# baseline (speedup 1.0000x reference)
"""PixelCrossAttention2D Trainium2 kernel.

Math (per sample b):
    xq = x @ W1 + b1            # [hw, F]
    yk = y @ W2 + b2            # [hw, F]
    A  = sigmoid(yk @ xq.T)     # [hw, hw]
    out = A @ yk + xq           # [hw, F]

Sharding: 8 cores = 4 samples x 2 row-halves. Core (b, h) computes output rows
[h*2048, (h+1)*2048) of sample b. Host rolls the sample's pixel axis by h*2048
so every core runs the identical SPMD program on rows 0:2048 (the j-sum over
all 4096 pixels is permutation invariant). Host also pre-transposes x/y to
feature-major [C, hw] bf16 (the layout/dtype every on-chip matmul wants) and
transposes the [F, 2048] per-core f32 result back.

On-chip per core:
    xqT = W1.T @ xT + b1        # [F, 4096]   (lhsT=W1, rhs=xT)
    ykT = W2.T @ yT + b2        # [F, 4096]
    yk_nat[t] = PE-transpose(ykT tile t)      # [j, F] per 128-pixel tile
    for ch in {0, 1}:           # i-columns 1024 per chunk
      for jt in 0..31:
        qk_psum[j=128, i=1024] = xqT[:, jt].T @ ykT[:, ch]      # logits.T
        at = sigmoid(qk_psum)   # ACT, PSUM -> SBUF, bf16
        av_psum[F, 1024] += yk_nat[jt].T @ at                   # out.T
      outT[:, ch] = av_psum + xqT[:, ch]       # residual

The scalar engine (64 sigmoid ops over PSUM, ~1.1us each) is the roofline;
input DMA/projection chunks are interleaved with the first half of the main
loop so the sigmoid stream starts as early as possible.
"""

import numpy as np

import concourse.bass as bass
import concourse.mybir as mybir
import concourse.tile as tile
from concourse import bacc, bass_utils
from concourse.masks import make_identity

F32 = mybir.dt.float32
BF16 = mybir.dt.bfloat16

N_CORES = 8
HW = 4096          # pixels per sample
NF = 128           # feature dim
I_ROWS = 2048      # output rows per core
I_CHUNK = 1024     # i-columns per PSUM chunk
N_JT = HW // 128   # 32 j-tiles
PRE_CHUNK = 2048   # preamble DMA/projection chunk width

_CACHE = {}


def _build():
    nc = bacc.Bacc("TRN2", target_bir_lowering=False, debug=False,
                   num_devices=N_CORES)
    # host pre-casts x/y to bf16 and pre-transposes to [C, hw]
    xT_d = nc.dram_tensor("xT", [128, HW], BF16, kind="ExternalInput")
    yT_d = nc.dram_tensor("yT", [128, HW], BF16, kind="ExternalInput")
    # packed [W1 | W2 | b1 | b2 | b1-row(row0)] -> one DMA trigger
    wcat_d = nc.dram_tensor("Wcat", [128, 3 * NF + 2], F32, kind="ExternalInput")
    outT_d = nc.dram_tensor("outT", [128, I_ROWS], F32, kind="ExternalOutput")

    SIG = mybir.ActivationFunctionType.Sigmoid

    with tile.TileContext(nc) as tc:
        with (
            tc.tile_pool(name="const", bufs=1) as cp,
            tc.tile_pool(name="big", bufs=1) as bp,
            tc.tile_pool(name="ob", bufs=4) as ob,
            tc.tile_pool(name="at", bufs=4) as atp,
            tc.tile_pool(name="psq", bufs=3, space="PSUM") as psq,
            tc.tile_pool(name="psav", bufs=1, space="PSUM") as psav,
        ):
            wcat = cp.tile([128, 3 * NF + 2], F32, tag="wcat")
            ident = cp.tile([128, 128], F32, tag="ident")
            identb = cp.tile([128, 128], BF16, tag="identb")
            sgdummy = cp.tile([128, 1], F32, tag="sgdummy")

            # preload the sigmoid ACT table while DMAs run
            nc.gpsimd.memset(sgdummy[:], 0.0)
            nc.scalar.activation(sgdummy[:], sgdummy[:], SIG)

            nc.sync.dma_start(wcat[:], wcat_d.ap())
            make_identity(nc, ident[:])
            nc.vector.tensor_copy(identb[:], ident[:])

            # ~3.5us of dummy matmuls: trip the PE HAM monitor to full clock
            # before the projections land on the critical path
            warmsrc = cp.tile([128, 512], BF16, tag="warmsrc")
            nc.gpsimd.memset(warmsrc[:], 0.0)
            pwarm = psq.tile([128, 512], F32, tag="qk")
            for _ in range(8):
                nc.tensor.matmul(pwarm[:], identb[:], warmsrc[:],
                                 start=True, stop=True)

            w1 = wcat[:, 0:NF]
            w2 = wcat[:, NF:2 * NF]
            b2 = wcat[:, 2 * NF + 1:2 * NF + 2]
            b1row = wcat[0:1, 2 * NF + 2:3 * NF + 2]   # b1 along row 0
            w1b = cp.tile([128, NF], BF16, tag="w1b")
            w2b = cp.tile([128, NF], BF16, tag="w2b")
            b1rb = cp.tile([1, NF], BF16, tag="b1rb")
            ones = cp.tile([1, 512], BF16, tag="ones")
            nc.vector.tensor_copy(w1b[:], w1)
            nc.vector.tensor_copy(w2b[:], w2)
            nc.vector.tensor_copy(b1rb[:], b1row)
            nc.gpsimd.memset(ones[:], 1.0)

            xTb = bp.tile([128, HW], BF16, tag="xTb")
            yTb = bp.tile([128, HW], BF16, tag="yTb")
            xqT = bp.tile([128, HW], BF16, tag="xqT")
            ykT = bp.tile([128, HW], BF16, tag="ykT")
            yk_nat = bp.tile([128, N_JT, 128], BF16, tag="yk_nat")

            # trigger every input DMA up front (2048-wide); data lands while
            # the warmup matmuls and first projections run
            for c in range(2):
                sl = bass.ts(c, 2048)
                nc.sync.dma_start(yTb[:, sl], yT_d.ap()[:, sl])
                nc.sync.dma_start(xTb[:, sl], xT_d.ap()[:, sl])

            COPY = mybir.ActivationFunctionType.Copy

            def preamble_piece(c):
                # project + transpose pixels [c*1024, (c+1)*1024)
                sl = bass.ts(c, 1024)
                # projections; ykT bias-add on DVE, xqT on ACT so the two
                # chains drain the PSUM slots in parallel
                py = psq.tile([128, 1024], F32, tag="qk")
                for s in range(2):
                    ssl = bass.ds(c * 1024 + s * 512, 512)
                    nc.tensor.matmul(py[:, bass.ts(s, 512)],
                                     w2b[:], yTb[:, ssl],
                                     start=True, stop=True)
                nc.vector.tensor_scalar_add(ykT[:, sl], py[:], b2)
                px = psq.tile([128, 1024], F32, tag="qk")
                for s in range(2):
                    ssl = bass.ds(c * 1024 + s * 512, 512)
                    nc.tensor.matmul(px[:, bass.ts(s, 512)],
                                     w1b[:], xTb[:, ssl],
                                     start=True, stop=False)
                    # + b1 (rank-1: b1-row outer ones), so the ACT copy
                    # below needs no bias operand
                    nc.tensor.matmul(px[:, bass.ts(s, 512)],
                                     b1rb[:], ones[:],
                                     start=False, stop=True)
                nc.scalar.activation(xqT[:, sl], px[:], COPY)
                # yk pixel-major tiles: 2-byte DMA xbar transpose, engine-free
                for t in range(c * 8, c * 8 + 8):
                    nc.sync.dma_start_transpose(
                        yk_nat[:, t, :], ykT[:, bass.ts(t, 128)])

            def main_jt(ch, av, jt):
                qk = psq.tile([128, I_CHUNK], F32, tag="qk")
                lhs = xqT[:, bass.ts(jt, 128)]
                for s in range(2):
                    sl = bass.ds(ch * I_CHUNK + s * 512, 512)
                    nc.tensor.matmul(qk[:, bass.ts(s, 512)], lhs, ykT[:, sl],
                                     start=True, stop=True)
                at = atp.tile([128, I_CHUNK], BF16, tag="at")
                nc.scalar.activation(at[:], qk[:], SIG)
                for s in range(2):
                    nc.tensor.matmul(
                        av[:, bass.ts(s, 512)],
                        yk_nat[:, jt, :], at[:, bass.ts(s, 512)],
                        start=(jt == 0), stop=(jt == N_JT - 1))

            def epilogue(ch, av):
                for s in range(2):
                    oT = ob.tile([128, 512], F32, tag="oT")
                    sl = bass.ds(ch * I_CHUNK + s * 512, 512)
                    nc.vector.tensor_add(oT[:], av[:, bass.ts(s, 512)],
                                         xqT[:, sl])
                    nc.sync.dma_start(outT_d.ap()[:, sl], oT[:])

            # all projections run before the sigmoid stream: the DVE bias-add
            # chain (~10us) finishes before the j-tiles that need it, and the
            # qk PSUM slots are never contended mid-stream
            for k in range(4):
                preamble_piece(k)
            av0 = psav.tile([128, I_CHUNK], F32, tag="av")
            for jt in range(N_JT):
                main_jt(0, av0, jt)
            epilogue(0, av0)
            av1 = psav.tile([128, I_CHUNK], F32, tag="av")
            for jt in range(N_JT):
                main_jt(1, av1, jt)
            epilogue(1, av1)

    nc.compile()
    return nc


def get_nc():
    if "nc" not in _CACHE:
        _CACHE["nc"] = _build()
    return _CACHE["nc"]


def make_in_maps(x, y, W1, b1, W2, b2):
    import ml_dtypes
    B, H, W, C = x.shape
    hw = H * W
    xf = np.ascontiguousarray(x, dtype=np.float32).reshape(B, hw, C)
    yf = np.ascontiguousarray(y, dtype=np.float32).reshape(B, hw, C)
    wcat = np.zeros((128, 3 * 128 + 2), np.float32)
    wcat[:, 0:128] = np.asarray(W1, np.float32)
    wcat[:, 128:256] = np.asarray(W2, np.float32)
    wcat[:, 256] = np.asarray(b1, np.float32).ravel()
    wcat[:, 257] = np.asarray(b2, np.float32).ravel()
    wcat[0, 258:386] = np.asarray(b1, np.float32).ravel()
    in_maps = []
    for core in range(N_CORES):
        b, h = divmod(core, 2)
        s = h * I_ROWS
        xr = np.roll(xf[b], -s, axis=0)
        yr = np.roll(yf[b], -s, axis=0)
        in_maps.append({
            "xT": np.ascontiguousarray(xr.T).astype(ml_dtypes.bfloat16),
            "yT": np.ascontiguousarray(yr.T).astype(ml_dtypes.bfloat16),
            "Wcat": wcat,
        })
    return in_maps


def run(inputs, trace=False):
    nc = get_nc()
    in_maps = make_in_maps(**inputs)
    try:
        res = bass_utils.run_bass_kernel_spmd(
            nc, in_maps, list(range(N_CORES)), trace=trace)
    except Exception:
        # transient NRT_EXEC_UNIT_UNRECOVERABLE wedges recover on retry
        res = bass_utils.run_bass_kernel_spmd(
            nc, in_maps, list(range(N_CORES)), trace=trace)
    x = inputs["x"]
    B, H, W, C = x.shape
    out = np.empty((B, H * W, NF), np.float32)
    for core in range(N_CORES):
        b, h = divmod(core, 2)
        out[b, h * I_ROWS:(h + 1) * I_ROWS, :] = res.results[core]["outT"].T
    return out.reshape(B, H, W, NF), res


def kernel(**inputs):
    out, _ = run(inputs, trace=False)
    return out


# revision 40
# speedup vs baseline: 1.0195x; 1.0195x over previous
"""PixelCrossAttention2D Trainium2 kernel.

Math (per sample b):
    xq = x @ W1 + b1            # [hw, F]
    yk = y @ W2 + b2            # [hw, F]
    A  = sigmoid(yk @ xq.T)     # [hw, hw]
    out = A @ yk + xq           # [hw, F]

Sharding: 8 cores = 4 samples x 2 row-halves. Core (b, h) computes output rows
[h*2048, (h+1)*2048) of sample b. Host rolls the sample's pixel axis by h*2048
so every core runs the identical SPMD program on rows 0:2048 (the j-sum over
all 4096 pixels is permutation invariant). Host also pre-transposes x/y to
feature-major [C, hw] bf16 (the layout/dtype every on-chip matmul wants) and
transposes the [F, 2048] per-core f32 result back.

On-chip per core:
    xqT = W1.T @ xT + b1        # [F, 4096]   (lhsT=W1, rhs=xT)
    ykT = W2.T @ yT + b2        # [F, 4096]
    yk_nat[t] = PE-transpose(ykT tile t)      # [j, F] per 128-pixel tile
    for ch in {0, 1}:           # i-columns 1024 per chunk
      for jt in 0..31:
        qk_psum[j=128, i=1024] = xqT[:, jt].T @ ykT[:, ch]      # logits.T
        at = sigmoid(qk_psum)   # ACT, PSUM -> SBUF, bf16
        av_psum[F, 1024] += yk_nat[jt].T @ at                   # out.T
      outT[:, ch] = av_psum + xqT[:, ch]       # residual

The scalar engine (64 sigmoid ops over PSUM, ~1.1us each) is the roofline;
input DMA/projection chunks are interleaved with the first half of the main
loop so the sigmoid stream starts as early as possible.
"""

import numpy as np

import concourse.bass as bass
import concourse.mybir as mybir
import concourse.tile as tile
from concourse import bacc, bass_utils
from concourse.masks import make_identity

F32 = mybir.dt.float32
BF16 = mybir.dt.bfloat16

N_CORES = 8
HW = 4096          # pixels per sample
NF = 128           # feature dim
I_ROWS = 2048      # output rows per core
I_CHUNK = 1024     # i-columns per PSUM chunk
N_JT = HW // 128   # 32 j-tiles
PRE_CHUNK = 2048   # preamble DMA/projection chunk width

_CACHE = {}


def _build():
    nc = bacc.Bacc("TRN2", target_bir_lowering=False, debug=False,
                   num_devices=N_CORES)
    # host pre-casts x/y to bf16 and pre-transposes to [C, hw]
    xT_d = nc.dram_tensor("xT", [128, HW], BF16, kind="ExternalInput")
    yT_d = nc.dram_tensor("yT", [128, HW], BF16, kind="ExternalInput")
    # packed [W1 | W2 | b1 | b2] -> one DMA trigger
    wcat_d = nc.dram_tensor("Wcat", [128, 2 * NF + 2], F32, kind="ExternalInput")
    outT_d = nc.dram_tensor("outT", [128, I_ROWS], F32, kind="ExternalOutput")

    SIG = mybir.ActivationFunctionType.Sigmoid

    with tile.TileContext(nc) as tc:
        with (
            tc.tile_pool(name="const", bufs=1) as cp,
            tc.tile_pool(name="big", bufs=1) as bp,
            tc.tile_pool(name="ob", bufs=4) as ob,
            tc.tile_pool(name="at", bufs=4) as atp,
            tc.tile_pool(name="psq", bufs=3, space="PSUM") as psq,
            tc.tile_pool(name="psav", bufs=1, space="PSUM") as psav,
        ):
            wcat = cp.tile([128, 2 * NF + 2], F32, tag="wcat")
            ident = cp.tile([128, 128], F32, tag="ident")
            identb = cp.tile([128, 128], BF16, tag="identb")
            sgdummy = cp.tile([128, 1], F32, tag="sgdummy")

            # preload the sigmoid ACT table while DMAs run
            nc.gpsimd.memset(sgdummy[:], 0.0)
            nc.scalar.activation(sgdummy[:], sgdummy[:], SIG)

            nc.sync.dma_start(wcat[:], wcat_d.ap())
            make_identity(nc, ident[:])
            nc.vector.tensor_copy(identb[:], ident[:])

            # ~5us of dummy matmuls: trip the PE HAM monitor to full clock
            # before the projections land on the critical path
            warmsrc = cp.tile([128, 512], BF16, tag="warmsrc")
            nc.gpsimd.memset(warmsrc[:], 0.0)
            pwarm = psq.tile([128, 512], F32, tag="qk")
            for _ in range(11):
                nc.tensor.matmul(pwarm[:], identb[:], warmsrc[:],
                                 start=True, stop=True)

            w1 = wcat[:, 0:NF]
            w2 = wcat[:, NF:2 * NF]
            b1 = wcat[:, 2 * NF:2 * NF + 1]
            b2 = wcat[:, 2 * NF + 1:2 * NF + 2]
            w1b = cp.tile([128, NF], BF16, tag="w1b")
            w2b = cp.tile([128, NF], BF16, tag="w2b")
            nc.vector.tensor_copy(w1b[:], w1)
            nc.vector.tensor_copy(w2b[:], w2)

            xTb = bp.tile([128, HW], BF16, tag="xTb")
            yTb = bp.tile([128, HW], BF16, tag="yTb")
            xqT = bp.tile([128, HW], BF16, tag="xqT")
            ykT = bp.tile([128, HW], BF16, tag="ykT")
            yk_nat = bp.tile([128, N_JT, 128], BF16, tag="yk_nat")

            # trigger every input DMA up front (2048-wide chunks keep the
            # SP trigger serialization short); data lands while the warmup
            # matmuls and first projections run
            for c in range(2):
                sl = bass.ts(c, 2048)
                nc.sync.dma_start(yTb[:, sl], yT_d.ap()[:, sl])
                nc.sync.dma_start(xTb[:, sl], xT_d.ap()[:, sl])

            def preamble_piece(c):
                # project + transpose pixels [c*1024, (c+1)*1024)
                sl = bass.ts(c, 1024)
                # projections, ykT first (feeds transposes)
                for w, srcb, b, dst in ((w2b, yTb, b2, ykT),
                                        (w1b, xTb, b1, xqT)):
                    p = psq.tile([128, 1024], F32, tag="qk")
                    for s in range(2):
                        ssl = bass.ds(c * 1024 + s * 512, 512)
                        nc.tensor.matmul(p[:, bass.ts(s, 512)],
                                         w[:], srcb[:, ssl],
                                         start=True, stop=True)
                    nc.vector.tensor_scalar_add(dst[:, sl], p[:], b)
                # yk pixel-major tiles: 2-byte DMA xbar transpose, engine-free
                for t in range(c * 8, c * 8 + 8):
                    nc.sync.dma_start_transpose(
                        yk_nat[:, t, :], ykT[:, bass.ts(t, 128)])

            def main_jt(ch, av, jt):
                qk = psq.tile([128, I_CHUNK], F32, tag="qk")
                lhs = xqT[:, bass.ts(jt, 128)]
                for s in range(2):
                    sl = bass.ds(ch * I_CHUNK + s * 512, 512)
                    nc.tensor.matmul(qk[:, bass.ts(s, 512)], lhs, ykT[:, sl],
                                     start=True, stop=True)
                at = atp.tile([128, I_CHUNK], BF16, tag="at")
                nc.scalar.activation(at[:], qk[:], SIG)
                for s in range(2):
                    nc.tensor.matmul(
                        av[:, bass.ts(s, 512)],
                        yk_nat[:, jt, :], at[:, bass.ts(s, 512)],
                        start=(jt == 0), stop=(jt == N_JT - 1))

            def epilogue(ch, av):
                for s in range(2):
                    oT = ob.tile([128, 512], F32, tag="oT")
                    sl = bass.ds(ch * I_CHUNK + s * 512, 512)
                    nc.vector.tensor_add(oT[:], av[:, bass.ts(s, 512)],
                                         xqT[:, sl])
                    nc.sync.dma_start(outT_d.ap()[:, sl], oT[:])

            # all projections run before the sigmoid stream: the DVE bias-add
            # chain (~10us) finishes before the j-tiles that need it, and the
            # qk PSUM slots are never contended mid-stream
            for k in range(4):
                preamble_piece(k)
            av0 = psav.tile([128, I_CHUNK], F32, tag="av")
            for jt in range(N_JT):
                main_jt(0, av0, jt)
            epilogue(0, av0)
            av1 = psav.tile([128, I_CHUNK], F32, tag="av")
            for jt in range(N_JT):
                main_jt(1, av1, jt)
            epilogue(1, av1)

    nc.compile()
    return nc


def get_nc():
    if "nc" not in _CACHE:
        _CACHE["nc"] = _build()
    return _CACHE["nc"]


def make_in_maps(x, y, W1, b1, W2, b2):
    import ml_dtypes
    B, H, W, C = x.shape
    hw = H * W
    xf = np.ascontiguousarray(x, dtype=np.float32).reshape(B, hw, C)
    yf = np.ascontiguousarray(y, dtype=np.float32).reshape(B, hw, C)
    wcat = np.concatenate(
        [np.asarray(W1, np.float32), np.asarray(W2, np.float32),
         np.asarray(b1, np.float32).reshape(128, 1),
         np.asarray(b2, np.float32).reshape(128, 1)], axis=1)
    wcat = np.ascontiguousarray(wcat)
    in_maps = []
    for core in range(N_CORES):
        b, h = divmod(core, 2)
        s = h * I_ROWS
        xr = np.roll(xf[b], -s, axis=0)
        yr = np.roll(yf[b], -s, axis=0)
        in_maps.append({
            "xT": np.ascontiguousarray(xr.T).astype(ml_dtypes.bfloat16),
            "yT": np.ascontiguousarray(yr.T).astype(ml_dtypes.bfloat16),
            "Wcat": wcat,
        })
    return in_maps


def run(inputs, trace=False):
    nc = get_nc()
    in_maps = make_in_maps(**inputs)
    try:
        res = bass_utils.run_bass_kernel_spmd(
            nc, in_maps, list(range(N_CORES)), trace=trace)
    except Exception:
        # transient NRT_EXEC_UNIT_UNRECOVERABLE wedges recover on retry
        res = bass_utils.run_bass_kernel_spmd(
            nc, in_maps, list(range(N_CORES)), trace=trace)
    x = inputs["x"]
    B, H, W, C = x.shape
    out = np.empty((B, H * W, NF), np.float32)
    for core in range(N_CORES):
        b, h = divmod(core, 2)
        out[b, h * I_ROWS:(h + 1) * I_ROWS, :] = res.results[core]["outT"].T
    return out.reshape(B, H, W, NF), res


def kernel(**inputs):
    out, _ = run(inputs, trace=False)
    return out
